# revision 32
# baseline (speedup 1.0000x reference)
"""Trainium2 Bass kernel for Bahdanau-style attention (nn_Attention).

Reference computation (B=128, S=1024, D=512):
    proj = tanh(concat(dec, enc) @ W1.T + b1)        # [B, S, D]
    scores = proj @ W2.T (+ b2, cancels in softmax)  # [B, S]
    alpha = softmax(scores, axis=1)
    context = einsum('bs,bsd->bd', alpha, enc)       # [B, D]

Strategy: pure data-parallel over batch (16 rows per NeuronCore, 8 cores).
Per-core dataflow (all matmuls bf16, fp32 PSUM accumulate):
  - hiddenT layout [h, s]: stationary = W1enc^T chunks, moving = enc^T tiles,
    so (proj_dec[b] + b1) becomes a per-partition bias fused into the
    ScalarE tanh that evacuates PSUM.
  - scores = W2 . hiddenT via PE matmuls with W2 column chunks as stationary.
  - softmax batched over groups of 4 batch rows on DVE/ScalarE
    (Exp with bias=-max and fused accum_out for the denominator).
  - alpha (normalized, bf16) transposed via PE transpose; context = alphaT^T @
    enc_natural via PE matmuls.
Host side: shard batch, pre-transpose/cast enc to both layouts in bf16.
"""

import numpy as np
import ml_dtypes

B, S, D = 128, 1024, 512
N_CORES = 8
B_LOC = B // N_CORES          # 16
GB = 4                        # batch rows per softmax group
NG = B_LOC // GB              # 4 groups
DC = D // 128                 # 4 chunks of 128 along d (and h)
SBLK = 512                    # s block for proj/score tiles
NSB = S // SBLK               # 2
NSC = S // 128                # 8 s-chunks of 128

_NPBF = ml_dtypes.bfloat16
_CACHE: dict = {}


def _build():
    from contextlib import ExitStack
    import concourse.bass as bass  # noqa: F401
    import concourse.tile as tile
    from concourse import bacc, mybir

    f32, bf16 = mybir.dt.float32, mybir.dt.bfloat16
    AX = mybir.AxisListType
    OP = mybir.AluOpType
    AF = mybir.ActivationFunctionType

    nc = bacc.Bacc("TRN2", target_bir_lowering=False, debug=False,
                   num_devices=N_CORES)

    encT = nc.dram_tensor("encT", [B_LOC, NSB, 128, DC, SBLK], bf16, kind="ExternalInput").ap()
    encN = nc.dram_tensor("encN", [B_LOC, 128, NSC, D], bf16, kind="ExternalInput").ap()
    w1eT = nc.dram_tensor("w1eT", [DC, 128, D], bf16, kind="ExternalInput").ap()
    w1dT = nc.dram_tensor("w1dT", [DC, 128, D], bf16, kind="ExternalInput").ap()
    decT = nc.dram_tensor("decT", [DC, 128, B_LOC], bf16, kind="ExternalInput").ap()
    b1c = nc.dram_tensor("b1c", [DC, 128, 1], f32, kind="ExternalInput").ap()
    w2c = nc.dram_tensor("w2c", [DC, 128, 1], bf16, kind="ExternalInput").ap()
    ident = nc.dram_tensor("ident", [128, 128], bf16, kind="ExternalInput").ap()
    out = nc.dram_tensor("out", [B_LOC, D], f32, kind="ExternalOutput").ap()

    with tile.TileContext(nc) as tc, ExitStack() as ctx:
        singles = ctx.enter_context(tc.tile_pool(name="singles", bufs=1))
        encT_pool = ctx.enter_context(tc.tile_pool(name="encTp", bufs=6))
        # Emission order drives DMA priority, and each dma_start costs
        # ~0.8us of serial descriptor-gen on its issuing engine -- so split
        # the first working set finely and spread issue across idle engines.
        w1eT_r = w1eT.rearrange("dc p h -> p dc h")
        w1dT_r = w1dT.rearrange("dc p h -> p dc h")

        # pd(hc0) needs only ~150KB; P(hc0) ~650KB. Front-load exactly those.
        w1d_slabs = []
        w1d_hc0 = singles.tile([128, DC, 128], bf16, name="w1d_hc0")
        nc.sync.dma_start(out=w1d_hc0, in_=w1dT_r[:, :, 0:128])
        w1d_slabs.append(w1d_hc0)
        dec_sb = singles.tile([128, DC, B_LOC], bf16)
        nc.sync.dma_start(out=dec_sb, in_=decT.rearrange("dc p b -> p dc b"))
        b1_sb = singles.tile([128, DC, 1], f32)
        nc.sync.dma_start(out=b1_sb, in_=b1c.rearrange("dc p o -> p dc o"))
        w1e_slabs = []
        w1e_hc0 = singles.tile([128, DC, 128], bf16, name="w1e_hc0")
        nc.sync.dma_start(out=w1e_hc0, in_=w1eT_r[:, :, 0:128])
        w1e_slabs.append(w1e_hc0)
        encT_b0s0 = encT_pool.tile([128, DC, SBLK], bf16, tag="encT")
        nc.sync.dma_start(out=encT_b0s0, in_=encT[0, 0])
        for hc in range(1, DC):
            w1d_hc = singles.tile([128, DC, 128], bf16, name=f"w1d_hc{hc}")
            nc.sync.dma_start(out=w1d_hc, in_=w1dT_r[:, :, hc * 128:(hc + 1) * 128])
            w1d_slabs.append(w1d_hc)
            w1e_hc = singles.tile([128, DC, 128], bf16, name=f"w1e_hc{hc}")
            nc.sync.dma_start(out=w1e_hc, in_=w1eT_r[:, :, hc * 128:(hc + 1) * 128])
            w1e_slabs.append(w1e_hc)
        encT_b0s1 = encT_pool.tile([128, DC, SBLK], bf16, tag="encT")
        nc.sync.dma_start(out=encT_b0s1, in_=encT[0, 1])
        w2_sb = singles.tile([128, DC, 1], bf16)
        nc.sync.dma_start(out=w2_sb, in_=w2c.rearrange("dc p o -> p dc o"))
        ident_sb = singles.tile([128, 128], bf16)
        nc.sync.dma_start(out=ident_sb, in_=ident)
        pdb1 = singles.tile([128, DC, B_LOC], f32)
        def emit_pd(hc):
            pd_ps = trctx_ps.tile([128, B_LOC], f32, tag="trctx", name=f"pd{hc}")
            for dc in range(DC):
                nc.tensor.matmul(
                    pd_ps,
                    lhsT=w1d_slabs[hc][:, dc, :],
                    rhs=dec_sb[:, dc, :],
                    start=(dc == 0), stop=(dc == DC - 1))
            nc.scalar.activation(out=pdb1[:, hc, :], in_=pd_ps,
                                 func=AF.Identity, bias=b1_sb[:, hc, :],
                                 scale=1.0)

        encN_pool = ctx.enter_context(tc.tile_pool(name="encNp", bufs=GB + 2))
        hT_pool = ctx.enter_context(tc.tile_pool(name="hTp", bufs=GB * NSB + 2))
        sg_pool = ctx.enter_context(tc.tile_pool(name="sgp", bufs=2))
        small = ctx.enter_context(tc.tile_pool(name="small", bufs=2))
        at_pool = ctx.enter_context(tc.tile_pool(name="atp", bufs=2))
        ctxg_pool = ctx.enter_context(tc.tile_pool(name="ctxgp", bufs=2))
        ph_pool = ctx.enter_context(tc.tile_pool(name="php", bufs=5, space="PSUM"))
        scsh_ps = ctx.enter_context(tc.tile_pool(name="scshps", bufs=2, space="PSUM"))
        trctx_ps = ctx.enter_context(tc.tile_pool(name="trctxps", bufs=1, space="PSUM"))

        for g in range(NG):
            # group rows live at partitions {0, 32, 64, 96}: engine writes to
            # a single partition are only legal at 32-aligned bases.
            scores_g = sg_pool.tile([128, S], f32, tag="scores")
            pmax = small.tile([128, NSB], f32, tag="pmax")
            encN_bs = []
            hT_units = {}
            for bi in range(GB):
                b = g * GB + bi
                if b == 0:
                    encT_sbs = [encT_b0s0, encT_b0s1]
                else:
                    encT_sbs = []
                    for sb in range(NSB):
                        encT_t = encT_pool.tile([128, DC, SBLK], bf16, tag="encT")
                        nc.sync.dma_start(out=encT_t, in_=encT[b, sb])
                        encT_sbs.append(encT_t)
                encN_b = encN_pool.tile([128, NSC, D], bf16, tag="encN")
                nc.sync.dma_start(out=encN_b, in_=encN[b])
                encN_bs.append(encN_b)
                for sb in range(NSB):
                    s0 = sb * SBLK
                    hT = hT_pool.tile([128, DC, SBLK], bf16, tag="hT")
                    hT_units[(bi, sb)] = hT
                    for hc in range(DC):
                        ph = ph_pool.tile([128, SBLK], f32, tag="ph")
                        for dc in range(DC):
                            nc.tensor.matmul(
                                ph,
                                lhsT=w1e_slabs[hc][:, dc, :],
                                rhs=encT_sbs[sb][:, dc, :],
                                start=(dc == 0), stop=(dc == DC - 1))
                        if b == 0 and sb == 0:
                            emit_pd(hc)
                        nc.scalar.activation(out=hT[:, hc, :], in_=ph,
                                             func=AF.Tanh,
                                             bias=pdb1[:, hc, b:b + 1],
                                             scale=1.0)

            # Batched scores: one col-tiled PSUM tile per s-block, batch row
            # bi lands at partition 32*bi. Keeping all M=1 matmuls in one
            # block avoids the ~100ns PE reconfig on M=128 <-> M=1 switches.
            for sb in range(NSB):
                sc_sh = scsh_ps.tile([128, SBLK], f32, tag="scsh",
                                     name=f"scsh{g}_{sb}")
                for bi in range(GB):
                    for hc in range(DC):
                        nc.tensor.matmul(sc_sh[32 * bi:32 * bi + 1, :],
                                         lhsT=w2_sb[:, hc, :],
                                         rhs=hT_units[(bi, sb)][:, hc, :],
                                         start=(hc == 0), stop=(hc == DC - 1),
                                         tile_position=(0, 32 * bi))
                s0 = sb * SBLK
                nc.vector.tensor_copy(out=scores_g[:, s0:s0 + SBLK], in_=sc_sh)
                nc.vector.tensor_reduce(out=pmax[:, sb:sb + 1], in_=sc_sh,
                                        axis=AX.X, op=OP.max)

            negmx = small.tile([128, 1], f32, tag="negmx")
            nc.vector.tensor_reduce(out=negmx, in_=pmax, axis=AX.X,
                                    op=OP.max, negate=True)
            nexp = 4
            estep = S // nexp
            scpere = NSC // nexp
            den_h = small.tile([128, NSC], f32, tag="den_h")
            alpha_n = sg_pool.tile([128, S], bf16, tag="alpha_n")
            alphaT = at_pool.tile([128, NSC, GB], bf16, tag="alphaT")
            for e in range(nexp):
                nc.scalar.activation(
                    out=alpha_n[:, e * estep:(e + 1) * estep],
                    in_=scores_g[:, e * estep:(e + 1) * estep], func=AF.Exp,
                    bias=negmx, scale=1.0,
                    accum_out=den_h[:, e:e + 1])
                for sc in range(e * scpere, (e + 1) * scpere):
                    tr_ps = trctx_ps.tile([128, 128], bf16, tag="trctx")
                    nc.tensor.transpose(tr_ps,
                                        alpha_n[:, sc * 128:(sc + 1) * 128],
                                        ident_sb)
                    # group rows sat at partitions 32*bi -> columns 32*bi
                    # after the transpose; gather into a dense tile.
                    nc.vector.tensor_copy(
                        out=alphaT[:, sc, :],
                        in_=tr_ps.rearrange("p (g r) -> p g r", g=GB)[:, :, 0])
            den = small.tile([128, 1], f32, tag="den")
            nc.vector.tensor_reduce(out=den, in_=den_h[:, 0:nexp], axis=AX.X,
                                    op=OP.add)
            rden = small.tile([128, 1], f32, tag="rden")
            nc.vector.reciprocal(out=rden, in_=den)

            ctx_g = ctxg_pool.tile([128, D], f32, tag="ctxg")
            ctx_sh = trctx_ps.tile([128, D], f32, tag="trctx",
                                   name=f"ctxsh{g}")
            for bi in range(GB):
                for sc in range(NSC):
                    nc.tensor.matmul(ctx_sh[32 * bi:32 * bi + 1, :],
                                     lhsT=alphaT[:, sc, bi:bi + 1],
                                     rhs=encN_bs[bi][:, sc, :],
                                     start=(sc == 0), stop=(sc == NSC - 1),
                                     tile_position=(0, 32 * bi))
            nc.vector.tensor_scalar(out=ctx_g, in0=ctx_sh, scalar1=rden,
                                    scalar2=None, op0=OP.mult)
            pitch = ctx_g.ap[0][0]
            ctx_rows = bass.AP(tensor=ctx_g.tensor, offset=ctx_g.offset,
                               ap=[[32 * pitch, GB]] +
                                  [list(dd) for dd in ctx_g.ap[1:]])
            nc.sync.dma_start(out=out[g * GB:(g + 1) * GB, :], in_=ctx_rows)

    nc.compile()
    return nc


def _get_nc():
    if "nc" not in _CACHE:
        _CACHE["nc"] = _build()
    return _CACHE["nc"]


def _prep_in_maps(inputs):
    dec = np.asarray(inputs["decoder_hidden"], dtype=np.float32)
    enc = np.asarray(inputs["encoder_outputs"], dtype=np.float32)
    W1 = np.asarray(inputs["W1"], dtype=np.float32)
    b1 = np.asarray(inputs["b1"], dtype=np.float32)
    W2 = np.asarray(inputs["W2"], dtype=np.float32)

    w1eT = np.ascontiguousarray(W1[:, D:].T).reshape(DC, 128, D).astype(_NPBF)
    w1dT = np.ascontiguousarray(W1[:, :D].T).reshape(DC, 128, D).astype(_NPBF)
    b1c = np.ascontiguousarray(b1).reshape(DC, 128, 1).astype(np.float32)
    w2c = np.ascontiguousarray(W2[0]).reshape(DC, 128, 1).astype(_NPBF)
    ident = np.eye(128, dtype=_NPBF)

    in_maps = []
    for c in range(N_CORES):
        sl = slice(c * B_LOC, (c + 1) * B_LOC)
        enc_c = enc[sl]                                  # [16, 1024, 512]
        # encT[b, sb, p, dc, s] = enc[b, sb*SBLK+s, dc*128+p] -- contiguous
        # 4KB per partition line per DMA.
        encT_c = np.ascontiguousarray(
            enc_c.reshape(B_LOC, NSB, SBLK, DC, 128).transpose(0, 1, 4, 3, 2)
        ).astype(_NPBF)
        # encN[b, p, sc, d] = enc[b, sc*128+p, d] -- contiguous 8KB lines.
        encN_c = np.ascontiguousarray(
            enc_c.reshape(B_LOC, NSC, 128, D).transpose(0, 2, 1, 3)
        ).astype(_NPBF)
        decT_c = np.ascontiguousarray(dec[sl].T).reshape(DC, 128, B_LOC) \
            .astype(_NPBF)
        in_maps.append({
            "encT": encT_c, "encN": encN_c, "w1eT": w1eT, "w1dT": w1dT,
            "decT": decT_c, "b1c": b1c, "w2c": w2c, "ident": ident,
        })
    return in_maps


def _run(inputs, trace=False, **kw):
    from concourse.bass_utils import run_bass_kernel_spmd
    nc = _get_nc()
    in_maps = _prep_in_maps(inputs)
    res = run_bass_kernel_spmd(nc, in_maps, core_ids=list(range(N_CORES)),
                               trace=trace, **kw)
    outs = [res.results[i]["out"] for i in range(N_CORES)]
    full = np.concatenate(outs, axis=0).astype(np.float32)
    return full, res


def kernel(**inputs) -> np.ndarray:
    full, _ = _run(inputs, trace=False)
    return full


# revision 33
# speedup vs baseline: 1.0301x; 1.0301x over previous
"""Trainium2 Bass kernel for Bahdanau-style attention (nn_Attention).

Reference computation (B=128, S=1024, D=512):
    proj = tanh(concat(dec, enc) @ W1.T + b1)        # [B, S, D]
    scores = proj @ W2.T (+ b2, cancels in softmax)  # [B, S]
    alpha = softmax(scores, axis=1)
    context = einsum('bs,bsd->bd', alpha, enc)       # [B, D]

Strategy: pure data-parallel over batch (16 rows per NeuronCore, 8 cores).
Per-core dataflow (all matmuls bf16, fp32 PSUM accumulate):
  - hiddenT layout [h, s]: stationary = W1enc^T chunks, moving = enc^T tiles,
    so (proj_dec[b] + b1) becomes a per-partition bias fused into the
    ScalarE tanh that evacuates PSUM.
  - scores = W2 . hiddenT via PE matmuls with W2 column chunks as stationary.
  - softmax batched over groups of 4 batch rows on DVE/ScalarE
    (Exp with bias=-max and fused accum_out for the denominator).
  - alpha (normalized, bf16) transposed via PE transpose; context = alphaT^T @
    enc_natural via PE matmuls.
Host side: shard batch, pre-transpose/cast enc to both layouts in bf16.
"""

import numpy as np
import ml_dtypes

B, S, D = 128, 1024, 512
N_CORES = 8
B_LOC = B // N_CORES          # 16
GB = 4                        # batch rows per softmax group
NG = B_LOC // GB              # 4 groups
DC = D // 128                 # 4 chunks of 128 along d (and h)
SBLK = 512                    # s block for proj/score tiles
NSB = S // SBLK               # 2
NSC = S // 128                # 8 s-chunks of 128

_NPBF = ml_dtypes.bfloat16
_CACHE: dict = {}


def _build():
    from contextlib import ExitStack
    import concourse.bass as bass  # noqa: F401
    import concourse.tile as tile
    from concourse import bacc, mybir

    f32, bf16 = mybir.dt.float32, mybir.dt.bfloat16
    AX = mybir.AxisListType
    OP = mybir.AluOpType
    AF = mybir.ActivationFunctionType

    nc = bacc.Bacc("TRN2", target_bir_lowering=False, debug=False,
                   num_devices=N_CORES)

    encT = nc.dram_tensor("encT", [B_LOC, NSB, 128, DC, SBLK], bf16, kind="ExternalInput").ap()
    encN = nc.dram_tensor("encN", [B_LOC, 128, NSC, D], bf16, kind="ExternalInput").ap()
    w1eT = nc.dram_tensor("w1eT", [DC, 128, DC, 128], bf16, kind="ExternalInput").ap()
    w1dT = nc.dram_tensor("w1dT", [DC, 128, DC, 128], bf16, kind="ExternalInput").ap()
    decT = nc.dram_tensor("decT", [DC, 128, B_LOC], bf16, kind="ExternalInput").ap()
    b1c = nc.dram_tensor("b1c", [DC, 128, 1], f32, kind="ExternalInput").ap()
    w2c = nc.dram_tensor("w2c", [DC, 128, 1], bf16, kind="ExternalInput").ap()
    ident = nc.dram_tensor("ident", [128, 128], bf16, kind="ExternalInput").ap()
    out = nc.dram_tensor("out", [B_LOC, D], f32, kind="ExternalOutput").ap()

    with tile.TileContext(nc) as tc, ExitStack() as ctx:
        singles = ctx.enter_context(tc.tile_pool(name="singles", bufs=1))
        encT_pool = ctx.enter_context(tc.tile_pool(name="encTp", bufs=6))
        # Emission order drives DMA priority, and each dma_start costs
        # ~0.8us of serial descriptor-gen on its issuing engine -- so split
        # the first working set finely and spread issue across idle engines.

        # pd(hc0) needs only ~150KB; P(hc0) ~650KB. Front-load exactly those.
        w1d_slabs = []
        w1d_hc0 = singles.tile([128, DC, 128], bf16, name="w1d_hc0")
        nc.sync.dma_start(out=w1d_hc0, in_=w1dT[0])
        w1d_slabs.append(w1d_hc0)
        dec_sb = singles.tile([128, DC, B_LOC], bf16)
        nc.sync.dma_start(out=dec_sb, in_=decT.rearrange("dc p b -> p dc b"))
        b1_sb = singles.tile([128, DC, 1], f32)
        nc.sync.dma_start(out=b1_sb, in_=b1c.rearrange("dc p o -> p dc o"))
        w1e_slabs = []
        w1e_hc0 = singles.tile([128, DC, 128], bf16, name="w1e_hc0")
        nc.sync.dma_start(out=w1e_hc0, in_=w1eT[0])
        w1e_slabs.append(w1e_hc0)
        encT_b0s0 = encT_pool.tile([128, DC, SBLK], bf16, tag="encT")
        nc.sync.dma_start(out=encT_b0s0, in_=encT[0, 0])
        for hc in range(1, DC):
            w1d_hc = singles.tile([128, DC, 128], bf16, name=f"w1d_hc{hc}")
            nc.sync.dma_start(out=w1d_hc, in_=w1dT[hc])
            w1d_slabs.append(w1d_hc)
            w1e_hc = singles.tile([128, DC, 128], bf16, name=f"w1e_hc{hc}")
            nc.sync.dma_start(out=w1e_hc, in_=w1eT[hc])
            w1e_slabs.append(w1e_hc)
        encT_b0s1 = encT_pool.tile([128, DC, SBLK], bf16, tag="encT")
        nc.sync.dma_start(out=encT_b0s1, in_=encT[0, 1])
        w2_sb = singles.tile([128, DC, 1], bf16)
        nc.sync.dma_start(out=w2_sb, in_=w2c.rearrange("dc p o -> p dc o"))
        ident_sb = singles.tile([128, 128], bf16)
        nc.sync.dma_start(out=ident_sb, in_=ident)
        pdb1 = singles.tile([128, DC, B_LOC], f32)
        def emit_pd(hc):
            pd_ps = trctx_ps.tile([128, B_LOC], f32, tag="trctx", name=f"pd{hc}")
            for dc in range(DC):
                nc.tensor.matmul(
                    pd_ps,
                    lhsT=w1d_slabs[hc][:, dc, :],
                    rhs=dec_sb[:, dc, :],
                    start=(dc == 0), stop=(dc == DC - 1))
            nc.scalar.activation(out=pdb1[:, hc, :], in_=pd_ps,
                                 func=AF.Identity, bias=b1_sb[:, hc, :],
                                 scale=1.0)

        encN_pool = ctx.enter_context(tc.tile_pool(name="encNp", bufs=GB + 2))
        hT_pool = ctx.enter_context(tc.tile_pool(name="hTp", bufs=GB * NSB + 2))
        sg_pool = ctx.enter_context(tc.tile_pool(name="sgp", bufs=2))
        small = ctx.enter_context(tc.tile_pool(name="small", bufs=2))
        at_pool = ctx.enter_context(tc.tile_pool(name="atp", bufs=2))
        ctxg_pool = ctx.enter_context(tc.tile_pool(name="ctxgp", bufs=2))
        ph_pool = ctx.enter_context(tc.tile_pool(name="php", bufs=5, space="PSUM"))
        scsh_ps = ctx.enter_context(tc.tile_pool(name="scshps", bufs=2, space="PSUM"))
        trctx_ps = ctx.enter_context(tc.tile_pool(name="trctxps", bufs=1, space="PSUM"))

        for g in range(NG):
            # group rows live at partitions {0, 32, 64, 96}: engine writes to
            # a single partition are only legal at 32-aligned bases.
            pmax = small.tile([128, NSB], f32, tag="pmax")
            sc_shs = []
            encN_bs = []
            hT_units = {}
            for bi in range(GB):
                b = g * GB + bi
                if b == 0:
                    encT_sbs = [encT_b0s0, encT_b0s1]
                else:
                    encT_sbs = []
                    for sb in range(NSB):
                        encT_t = encT_pool.tile([128, DC, SBLK], bf16, tag="encT")
                        nc.sync.dma_start(out=encT_t, in_=encT[b, sb])
                        encT_sbs.append(encT_t)
                encN_b = encN_pool.tile([128, NSC, D], bf16, tag="encN")
                nc.sync.dma_start(out=encN_b, in_=encN[b])
                encN_bs.append(encN_b)
                for sb in range(NSB):
                    s0 = sb * SBLK
                    hT = hT_pool.tile([128, DC, SBLK], bf16, tag="hT")
                    hT_units[(bi, sb)] = hT
                    for hc in range(DC):
                        ph = ph_pool.tile([128, SBLK], f32, tag="ph")
                        for dc in range(DC):
                            nc.tensor.matmul(
                                ph,
                                lhsT=w1e_slabs[hc][:, dc, :],
                                rhs=encT_sbs[sb][:, dc, :],
                                start=(dc == 0), stop=(dc == DC - 1))
                        if b == 0 and sb == 0:
                            emit_pd(hc)
                        nc.scalar.activation(out=hT[:, hc, :], in_=ph,
                                             func=AF.Tanh,
                                             bias=pdb1[:, hc, b:b + 1],
                                             scale=1.0)

            # Batched scores: one col-tiled PSUM tile per s-block, batch row
            # bi lands at partition 32*bi. Keeping all M=1 matmuls in one
            # block avoids the ~100ns PE reconfig on M=128 <-> M=1 switches.
            for sb in range(NSB):
                sc_sh = scsh_ps.tile([128, SBLK], f32, tag="scsh",
                                     name=f"scsh{g}_{sb}")
                for bi in range(GB):
                    for hc in range(DC):
                        nc.tensor.matmul(sc_sh[32 * bi:32 * bi + 1, :],
                                         lhsT=w2_sb[:, hc, :],
                                         rhs=hT_units[(bi, sb)][:, hc, :],
                                         start=(hc == 0), stop=(hc == DC - 1),
                                         tile_position=(0, 32 * bi))
                sc_shs.append(sc_sh)
                nc.vector.tensor_reduce(out=pmax[:, sb:sb + 1], in_=sc_sh,
                                        axis=AX.X, op=OP.max)

            negmx = small.tile([128, 1], f32, tag="negmx")
            nc.vector.tensor_reduce(out=negmx, in_=pmax, axis=AX.X,
                                    op=OP.max, negate=True)
            nexp = NSB
            estep = S // nexp
            scpere = NSC // nexp
            den_h = small.tile([128, NSC], f32, tag="den_h")
            alpha_n = sg_pool.tile([128, S], bf16, tag="alpha_n")
            alphaT = at_pool.tile([128, NSC, GB], bf16, tag="alphaT")
            for e in range(nexp):
                nc.scalar.activation(
                    out=alpha_n[:, e * estep:(e + 1) * estep],
                    in_=sc_shs[e], func=AF.Exp,
                    bias=negmx, scale=1.0,
                    accum_out=den_h[:, e:e + 1])
                for sc in range(e * scpere, (e + 1) * scpere):
                    tr_ps = trctx_ps.tile([128, 128], bf16, tag="trctx")
                    nc.tensor.transpose(tr_ps,
                                        alpha_n[:, sc * 128:(sc + 1) * 128],
                                        ident_sb)
                    # group rows sat at partitions 32*bi -> columns 32*bi
                    # after the transpose; gather into a dense tile.
                    nc.vector.tensor_copy(
                        out=alphaT[:, sc, :],
                        in_=tr_ps.rearrange("p (g r) -> p g r", g=GB)[:, :, 0])
            den = small.tile([128, 1], f32, tag="den")
            nc.vector.tensor_reduce(out=den, in_=den_h[:, 0:nexp], axis=AX.X,
                                    op=OP.add)
            rden = small.tile([128, 1], f32, tag="rden")
            nc.vector.reciprocal(out=rden, in_=den)

            ctx_g = ctxg_pool.tile([128, D], f32, tag="ctxg")
            ctx_sh = trctx_ps.tile([128, D], f32, tag="trctx",
                                   name=f"ctxsh{g}")
            for bi in range(GB):
                for sc in range(NSC):
                    nc.tensor.matmul(ctx_sh[32 * bi:32 * bi + 1, :],
                                     lhsT=alphaT[:, sc, bi:bi + 1],
                                     rhs=encN_bs[bi][:, sc, :],
                                     start=(sc == 0), stop=(sc == NSC - 1),
                                     tile_position=(0, 32 * bi))
            nc.vector.tensor_scalar(out=ctx_g, in0=ctx_sh, scalar1=rden,
                                    scalar2=None, op0=OP.mult)
            pitch = ctx_g.ap[0][0]
            ctx_rows = bass.AP(tensor=ctx_g.tensor, offset=ctx_g.offset,
                               ap=[[32 * pitch, GB]] +
                                  [list(dd) for dd in ctx_g.ap[1:]])
            nc.sync.dma_start(out=out[g * GB:(g + 1) * GB, :], in_=ctx_rows)

    nc.compile()
    return nc


def _get_nc():
    if "nc" not in _CACHE:
        _CACHE["nc"] = _build()
    return _CACHE["nc"]


def _prep_in_maps(inputs):
    dec = np.asarray(inputs["decoder_hidden"], dtype=np.float32)
    enc = np.asarray(inputs["encoder_outputs"], dtype=np.float32)
    W1 = np.asarray(inputs["W1"], dtype=np.float32)
    b1 = np.asarray(inputs["b1"], dtype=np.float32)
    W2 = np.asarray(inputs["W2"], dtype=np.float32)

    def _slab(wT):
        # wT [d, h] -> [hc, p, dc, h'] with d = dc*128+p, h = hc*128+h'
        return np.ascontiguousarray(
            wT.reshape(DC, 128, DC, 128).transpose(2, 1, 0, 3)).astype(_NPBF)

    w1eT = _slab(W1[:, D:].T)
    w1dT = _slab(W1[:, :D].T)
    b1c = np.ascontiguousarray(b1).reshape(DC, 128, 1).astype(np.float32)
    w2c = np.ascontiguousarray(W2[0]).reshape(DC, 128, 1).astype(_NPBF)
    ident = np.eye(128, dtype=_NPBF)

    in_maps = []
    for c in range(N_CORES):
        sl = slice(c * B_LOC, (c + 1) * B_LOC)
        enc_c = enc[sl]                                  # [16, 1024, 512]
        # encT[b, sb, p, dc, s] = enc[b, sb*SBLK+s, dc*128+p] -- contiguous
        # 4KB per partition line per DMA.
        encT_c = np.ascontiguousarray(
            enc_c.reshape(B_LOC, NSB, SBLK, DC, 128).transpose(0, 1, 4, 3, 2)
        ).astype(_NPBF)
        # encN[b, p, sc, d] = enc[b, sc*128+p, d] -- contiguous 8KB lines.
        encN_c = np.ascontiguousarray(
            enc_c.reshape(B_LOC, NSC, 128, D).transpose(0, 2, 1, 3)
        ).astype(_NPBF)
        decT_c = np.ascontiguousarray(dec[sl].T).reshape(DC, 128, B_LOC) \
            .astype(_NPBF)
        in_maps.append({
            "encT": encT_c, "encN": encN_c, "w1eT": w1eT, "w1dT": w1dT,
            "decT": decT_c, "b1c": b1c, "w2c": w2c, "ident": ident,
        })
    return in_maps


def _run(inputs, trace=False, **kw):
    from concourse.bass_utils import run_bass_kernel_spmd
    nc = _get_nc()
    in_maps = _prep_in_maps(inputs)
    res = run_bass_kernel_spmd(nc, in_maps, core_ids=list(range(N_CORES)),
                               trace=trace, **kw)
    outs = [res.results[i]["out"] for i in range(N_CORES)]
    full = np.concatenate(outs, axis=0).astype(np.float32)
    return full, res


def kernel(**inputs) -> np.ndarray:
    full, _ = _run(inputs, trace=False)
    return full
